# revision 23
# baseline (speedup 1.0000x reference)
"""Bidirectional Mamba selective scan on 8 Trainium2 NeuronCores.

Sharding: core c -> (batch b = c//2, d_inner half = c%2). Each core receives
x[b] pre-transposed to [D, L] with its own d-half rows first, computes the
(replicated, small) x_proj and dt_proj matmuls locally, and runs both scan
directions fully on-core: zero cross-core communication, one SPMD NEFF.

Per-core dataflow (Lc-chunked along L, d on partitions, s-major state dim):
  PE : dbc = x_proj_w @ x  (K=1024, out [64, Lc]); delta matmul (K=32)
  ACT: softplus(delta) with per-partition bias; exp(delta * A[:, s]) via
       per-partition scale; PSUM->SBUF copies
  DVE: BX = (delta*x) * B_bcast; tensor_tensor_scan (h = dA*h + BX);
       h *= C_bcast in place; strided tensor_reduce over s
  DMA: x / weights load, B/C partition-broadcast, y store
The backward direction reuses the same per-position pipeline; only the scan
runs on reversed (negative stride) access patterns with chunks visited
last-first.
"""

import numpy as np

import bass_rust
import concourse.bass as bass
import concourse.mybir as mybir
import concourse.tile as tile
from concourse.bass_utils import run_bass_kernel_spmd
from concourse.vector_clock import ScopedClock

F32 = mybir.dt.float32
OP = mybir.AluOpType
AF = mybir.ActivationFunctionType

B, L, DI, S, R = 4, 2048, 1024, 16, 32
DH = DI // 2          # d channels per core
NK = DI // 128        # K-chunks for the dbc matmul
NT = DH // 128        # d-tiles per core
LC = 512              # L chunk
NCH = L // LC
NB = R + 2 * S        # dbc rows (64)


class SplitDrainTileContext(tile.TileContext):
    """TileContext whose exit drain splits sem waits across instructions.

    This walrus build rejects instructions carrying >2 sync-wait commands
    ("Too many sync wait commands" in CoreV3 codegen). Stock TileContext
    attaches one wait per outstanding proc to the single final SP drain;
    emit one wait-carrier nop per proc instead.
    """

    def _drain_and_barrier(self, tick_clock, wait_clock):
        ticks = list(tick_clock.global_clock)
        self.nc.sync.drain()
        for i, t in enumerate(ticks):
            if t > 0:
                partial = bass_rust.VectorClock(
                    [t if j == i else 0 for j in range(len(ticks))]
                )
                carrier = self.nc.sync.nop(nofuse=True, hint="split_drain_wait")
                wait_clock.add_sem_waits(carrier.ins, ScopedClock({None: partial}))

        self.nc.all_engine_barrier()
        assert self.sems is not None
        popped = self.nc._tile_sem_poison_stack.pop()
        assert popped is self._sem_poison
        self.nc.clear_and_free_semaphores(list(self.sems.allocated().values()))
        self.nc.all_engine_barrier()


MAX_WAITS = 1  # sync-wait commands this walrus accepts per instruction


def legalize_sync_waits(json_bytes):
    """Split >cap on_wait conditions onto EventSemaphore carriers.

    This walrus build errors with "Too many sync wait commands" when one
    instruction carries more than `cap` waits. Hoist the excess onto
    same-engine EventSemaphore instructions inserted just before; engine
    program order makes the waits still happen-before the instruction
    (for DMAs: before descriptor enqueue).
    """
    import json

    m = json.loads(json_bytes)
    for f in m["functions"]:
        for bb in f["blocks"]:
            out = []
            changed = False
            for inst in bb["instructions"]:
                si = inst.get("sync_info") or {}
                ws = si.get("on_wait") or []
                cap = MAX_WAITS
                if len(ws) > cap:
                    changed = True
                    keep = ws[:cap]
                    rest = ws[cap:]
                    for i in range(0, len(rest), cap):
                        out.append({
                            "debug": inst.get("debug", 0),
                            "engine": inst["engine"],
                            "ins": [],
                            "name": f"{inst['name']}_w{i}",
                            "opcode": "EventSemaphore",
                            "outs": [],
                            "sync_info": {
                                "on_update": [],
                                "on_wait": rest[i:i + cap],
                            },
                        })
                    si["on_wait"] = keep
                    inst["sync_info"] = si
                out.append(inst)
            if changed:
                bb["instructions"] = out
    return json.dumps(m).encode()


def _bcast_ap(row_ap, parts=128):
    """View a single-partition row AP as a partition-stride-0 broadcast."""
    return bass.AP(
        tensor=row_ap.tensor,
        offset=row_ap.offset,
        ap=[[0, parts]] + [list(d) for d in row_ap.ap[1:]],
    )


def build_nc(repeat=1):
    nc = bass.Bass()

    xT = nc.dram_tensor("xT", [DI, L], F32, kind="ExternalInput")
    wdbc = nc.dram_tensor("wdbc", [2, DI, NB], F32, kind="ExternalInput")
    wdt = nc.dram_tensor("wdt", [2, R, DH], F32, kind="ExternalInput")
    bdt = nc.dram_tensor("bdt", [2, DH, 1], F32, kind="ExternalInput")
    Adr = nc.dram_tensor("A", [2, DH, S], F32, kind="ExternalInput")
    dsum = nc.dram_tensor("dsum", [DH, 1], F32, kind="ExternalInput")
    yT = nc.dram_tensor("yT", [DH, L], F32, kind="ExternalOutput")

    with SplitDrainTileContext(nc) as tc:
        with (
            tc.tile_pool(name="persist", bufs=1) as persist,
            tc.tile_pool(name="xs", bufs=2) as xs_pool,
            tc.tile_pool(name="dbc_ps", bufs=2, space="PSUM") as dbc_ps,
            tc.tile_pool(name="delta_ps", bufs=2, space="PSUM") as delta_ps,
            tc.tile_pool(name="dbc", bufs=2) as dbc_pool,
            tc.tile_pool(name="delta", bufs=2) as delta_pool,
            tc.tile_pool(name="u", bufs=2) as u_pool,
            tc.tile_pool(name="bc", bufs=1) as bc_pool,
            tc.tile_pool(name="da", bufs=2) as da_pool,
            tc.tile_pool(name="bx", bufs=3) as bx_pool,
            tc.tile_pool(name="h", bufs=1) as h_pool,
            tc.tile_pool(name="bcd", bufs=2, space="DRAM") as bcd_pool,
        ):
            # ---- persistent loads ----
            wdbc_sb = [[None] * NK for _ in range(2)]
            wdt_sb = [None] * 2
            bdt_sb = [[None] * NT for _ in range(2)]
            A_sb = [[None] * NT for _ in range(2)]
            for d in range(2):
                for k in range(NK):
                    w = persist.tile([128, NB], F32, tag=f"wdbc{d}_{k}")
                    nc.sync.dma_start(out=w[:, :], in_=wdbc[d, k * 128:(k + 1) * 128, :])
                    wdbc_sb[d][k] = w
                wt = persist.tile([R, DH], F32, tag=f"wdt{d}")
                nc.sync.dma_start(out=wt[:, :], in_=wdt[d, :, :])
                wdt_sb[d] = wt
                for t in range(NT):
                    bb = persist.tile([128, 1], F32, tag=f"bdt{d}_{t}")
                    nc.sync.dma_start(out=bb[:, :], in_=bdt[d, t * 128:(t + 1) * 128, :])
                    bdt_sb[d][t] = bb
                    aa = persist.tile([128, S], F32, tag=f"A{d}_{t}")
                    nc.sync.dma_start(out=aa[:, :], in_=Adr[d, t * 128:(t + 1) * 128, :])
                    A_sb[d][t] = aa

            dsum_sb = []
            for t in range(NT):
                dd = persist.tile([128, 1], F32, tag=f"dsum{t}")
                nc.sync.dma_start(out=dd[:, :], in_=dsum[t * 128:(t + 1) * 128, :])
                dsum_sb.append(dd)

            y_acc = [persist.tile([128, L], F32, tag=f"yacc{t}", name=f"yacc{t}")
                     for t in range(NT)]
            state_sb = [persist.tile([128, S], F32, tag=f"state{t}", name=f"state{t}")
                        for t in range(NT)]

            # ---- main loop ----
            for _rep in range(repeat):
              for d in range(2):
                fwd = d == 0
                chunks = range(NCH) if fwd else range(NCH - 1, -1, -1)
                first_ci = 0 if fwd else NCH - 1
                for ci in chunks:
                    lsl = slice(ci * LC, (ci + 1) * LC)

                    # stream this chunk's full-D xT columns (own half = 0..3)
                    xs = xs_pool.tile([128, NK, LC], F32)
                    for k in range(NK):
                        nc.sync.dma_start(
                            out=xs[:, k, :], in_=xT[k * 128:(k + 1) * 128, lsl]
                        )

                    # dbc = x_proj_w @ x : [64, LC]
                    ps = dbc_ps.tile([NB, LC], F32)
                    for k in range(NK):
                        nc.tensor.matmul(
                            ps[:, :], wdbc_sb[d][k][:, :], xs[:, k, :],
                            start=(k == 0), stop=(k == NK - 1),
                        )
                    dbc_sb = dbc_pool.tile([NB, LC], F32)
                    nc.scalar.copy(out=dbc_sb[:, :], in_=ps[:, :])

                    # broadcast B and C rows across partitions: bounce the 32
                    # rows through DRAM (stride-0 partition APs are DRAM-only)
                    bc_dram = bcd_pool.tile([2 * S, LC], F32, tag="bcd", name="bc_dram")
                    nc.sync.dma_start(out=bc_dram[:, :], in_=dbc_sb[R:NB, :])
                    bc_b = bc_pool.tile([128, S, LC], F32, tag="bcB", name="bc_b")
                    bc_c = bc_pool.tile([128, S, LC], F32, tag="bcC", name="bc_c")
                    for s in range(S):
                        nc.sync.dma_start(
                            out=bc_b[:, s, :],
                            in_=_bcast_ap(bc_dram[s:s + 1, :]),
                        )
                        nc.sync.dma_start(
                            out=bc_c[:, s, :],
                            in_=_bcast_ap(bc_dram[S + s:S + s + 1, :]),
                        )

                    for t in range(NT):
                        # delta = softplus(dt_w @ dbc_delta + bias) : [128, LC]
                        dps = delta_ps.tile([128, LC], F32)
                        nc.tensor.matmul(
                            dps[:, :], wdt_sb[d][:, t * 128:(t + 1) * 128],
                            dbc_sb[0:R, :], start=True, stop=True,
                        )
                        # softplus(z) = ln(exp(z) + 1): this walrus has no
                        # Softplus table, but Exp and Ln share one func set
                        # (natural_log_exp_and_others) => no table switches.
                        ez = da_pool.tile([128, LC], F32, tag="da", name="ez")
                        nc.scalar.activation(
                            out=ez[:, :], in_=dps[:, :], func=AF.Exp,
                            bias=bdt_sb[d][t][:, :], scale=1.0,
                        )
                        delta = delta_pool.tile([128, LC], F32)
                        nc.scalar.activation(
                            out=delta[:, :], in_=ez[:, :], func=AF.Ln,
                            bias=1.0, scale=1.0,
                        )
                        # u = delta * x  (own-half rows are xs tiles 0..3)
                        u = u_pool.tile([128, LC], F32)
                        nc.gpsimd.tensor_tensor(
                            out=u[:, :], in0=delta[:, :],
                            in1=xs[:, t, :], op=OP.mult,
                        )

                        h_big = h_pool.tile([128, S, LC], F32)
                        for s in range(S):
                            da = da_pool.tile([128, LC], F32)
                            nc.scalar.activation(
                                out=da[:, :], in_=delta[:, :], func=AF.Exp,
                                scale=A_sb[d][t][:, s:s + 1],
                            )
                            bx = bx_pool.tile([128, LC], F32)
                            nc.vector.tensor_tensor(
                                out=bx[:, :], in0=u[:, :], in1=bc_b[:, s, :],
                                op=OP.mult,
                            )
                            init = (
                                0.0 if ci == first_ci else state_sb[t][:, s:s + 1]
                            )
                            if fwd:
                                nc.vector.tensor_tensor_scan(
                                    out=h_big[:, s, :], data0=da[:, :],
                                    data1=bx[:, :], initial=init,
                                    op0=OP.mult, op1=OP.add,
                                )
                            else:
                                nc.vector.tensor_tensor_scan(
                                    out=h_big[:, s, :][:, ::-1],
                                    data0=da[:, :][:, ::-1],
                                    data1=bx[:, :][:, ::-1], initial=init,
                                    op0=OP.mult, op1=OP.add,
                                )

                        # save carry state (last processed column per s)
                        col = LC - 1 if fwd else 0
                        nc.vector.tensor_copy(
                            out=state_sb[t][:, :], in_=h_big[:, :, col],
                        )
                        # h *= C in one flat in-place multiply, then sum over
                        # s by folding the s range in half (unit-stride adds)
                        nc.vector.tensor_tensor(
                            out=h_big[:, :, :], in0=h_big[:, :, :],
                            in1=bc_c[:, :, :], op=OP.mult,
                        )
                        w = S
                        while w > 1:
                            w //= 2
                            nc.vector.tensor_tensor(
                                out=h_big[:, 0:w, :], in0=h_big[:, 0:w, :],
                                in1=h_big[:, w:2 * w, :], op=OP.add,
                            )
                        ysum = h_big[:, 0, :]
                        ysl = y_acc[t][:, lsl]
                        if fwd:
                            # y = (D + D_b) * x + scan_sum
                            nc.vector.scalar_tensor_tensor(
                                out=ysl, in0=xs[:, t, :],
                                scalar=dsum_sb[t][:, :], in1=ysum,
                                op0=OP.mult, op1=OP.add,
                            )
                        else:
                            nc.vector.tensor_tensor(
                                out=ysl, in0=ysl, in1=ysum, op=OP.add,
                            )
                            # chunk complete: store
                            nc.sync.dma_start(
                                out=yT[t * 128:(t + 1) * 128, lsl], in_=ysl,
                            )

    return nc


_NC_CACHE = []
TRACE = False
LAST_EXEC_NS = None
LAST_RESULTS = None


def _get_nc():
    if not _NC_CACHE:
        nc = build_nc()
        legal = legalize_sync_waits(nc.to_json_bytes())
        nc.to_json_bytes = lambda: legal
        _NC_CACHE.append(nc)
    return _NC_CACHE[0]


def kernel(x, x_proj_w, dt_proj_w, dt_proj_b, A_log, D,
           x_proj_b_w, dt_proj_b_w, dt_proj_b_b, A_b_log, D_b):
    x = np.asarray(x, np.float32)
    wdbc_full = np.stack(
        [np.asarray(x_proj_w, np.float32).T, np.asarray(x_proj_b_w, np.float32).T]
    )  # [2, DI, 64]
    wdt_full = np.stack(
        [np.asarray(dt_proj_w, np.float32).T, np.asarray(dt_proj_b_w, np.float32).T]
    )  # [2, R, DI]
    bdt_full = np.stack(
        [np.asarray(dt_proj_b, np.float32), np.asarray(dt_proj_b_b, np.float32)]
    )  # [2, DI]
    A_full = np.stack(
        [-np.exp(np.asarray(A_log, np.float32)),
         -np.exp(np.asarray(A_b_log, np.float32))]
    )  # [2, DI, S]
    dsum_full = np.asarray(D, np.float32) + np.asarray(D_b, np.float32)

    # Per half: permute d so the core's own half comes first; the dbc
    # matmul contracts over all of d, so weights get the same row permute.
    perm = [np.r_[0:DI], np.r_[DH:DI, 0:DH]]
    in_maps = []
    half_common = []
    for half in range(2):
        p = perm[half]
        ds = half * DH
        half_common.append({
            "wdbc": np.ascontiguousarray(wdbc_full[:, p, :]),
            "wdt": np.ascontiguousarray(wdt_full[:, :, ds:ds + DH]),
            "bdt": np.ascontiguousarray(bdt_full[:, ds:ds + DH, None]),
            "A": np.ascontiguousarray(A_full[:, ds:ds + DH, :]),
            "dsum": np.ascontiguousarray(dsum_full[ds:ds + DH, None]),
        })
    for c in range(8):
        b, half = c // 2, c % 2
        xTb = np.ascontiguousarray(x[b].T[perm[half], :])
        in_maps.append(dict(half_common[half], xT=xTb))

    nc = _get_nc()
    global LAST_EXEC_NS, LAST_RESULTS
    res = run_bass_kernel_spmd(
        nc, in_maps, core_ids=list(range(8)), trace=TRACE,
        trace_cores=list(range(8)) if TRACE else None,
    )
    LAST_EXEC_NS = res.exec_time_ns
    LAST_RESULTS = res

    y = np.empty((B, L, DI), np.float32)
    for c in range(8):
        b, half = c // 2, c % 2
        ds = half * DH
        y[b, :, ds:ds + DH] = res.results[c]["yT"].T
    return y
